# revision 8
# baseline (speedup 1.0000x reference)
"""DN4 retrieval-kNN layer as a Trainium2 Bass/Tile kernel (fp8 DoubleRow).

Reference computation (shapes hardcoded from the problem spec):
  query_feat  [t=4, wq=75, c=640, 10, 10]  -> q normalized over hw axis (per (wq, c))
  support_feat[t=4, ws=25, c=640, 10, 10]  -> s normalized over c axis (per (way, y))
  relation[t, wq, way, x, y] = sum_c qn[t, wq, x, c] * sn[t, way, c, y]   (x=100, y=500)
  score[t, wq, way] = sum_x sum(top3_y(relation))

Sharding: 8 cores = 4 episodes (t) x 2 query-halves. Core 2t handles queries
[0:38), core 2t+1 handles queries [37:75) (38 rows each; query 37 is computed
twice and deduplicated on the host). No cross-device communication.

Device kernel (per core):
  - host prep: inputs pre-transposed to [c, n, x] bf16; the segment matrix for
    the per-query row sum carries the 1/(SQ*SS) fp8 descale.
  - query normalize: per-query ACT Square with accum_out gives sum-of-squares
    over hw without touching DVE; DVE reciprocal + ACT sqrt (fp8 scale folded
    in); GPSIMD multiplies apply the normalizer and emit fp8e4m3 directly into
    the DoubleRow pair layout. Queries are processed in blocks so the main
    loop can start after the first block.
  - support normalize: ACT squares, ones-matmul partition reduce (PE), DVE
    reciprocal + ACT sqrt, ones outer-product broadcast (PE), DVE/GPSIMD
    multiplies emit fp8.
  - main loop over 30 groups of 128 flattened (query, x) rows: per way, 2
    fp8 DoubleRow matmuls (256-deep contraction each) + 1 plain fp8 matmul
    accumulate the [128, 500] relation tile in PSUM at 2x bf16 throughput;
    DVE max8 extracts top-8 per row; a segment-matrix matmul (delayed two
    groups to keep the in-order PE queue from stalling on DVE) accumulates
    all 40 way/top8 lanes into PSUM; one final strided reduce sums top-3.
"""

import sys
import numpy as np

sys.path.insert(0, "/opt/trn_rl_repo")

T, WQ, C, HW = 4, 75, 640, 100
WAY, SHOT = 5, 5
NS = WAY * SHOT          # 25 support images per episode
Y = SHOT * HW            # 500 support descriptors per way
YALL = WAY * Y           # 2500
QPC = 38                 # queries per core (overlapping halves of 75)
KC = C // 128            # 5 contraction chunks of 128
NCORES = 8
NK = 3                   # top-k
ROWS = QPC * HW          # 3800 flattened (query, x) relation rows per core
GROUPS = (ROWS + 127) // 128   # 30 row-groups of <=128
BK = 512                 # PSUM bank stride in fp32 elements
SQ = 16.0                # fp8 scale on normalized query
SS = 16.0                # fp8 scale on normalized support
SEG_DELAY = 2            # groups to delay the seg matmul behind max8
QBLOCKS = [2, 4, 8, 8, 8, 8]   # query pipeline blocks (sum = QPC)
S_MUL_ENG = "VVVPP"      # engine per way for the support scale-mul
RPAD = GROUPS * 128      # 3840: q8 pair stride must be 16-aligned (dual-fp8 ISA)
YPAD = 512               # s8 way stride, keeps the pair stride 16-aligned

_PROGRAM = None


def _build_program(phases=3, loop_reps=0, loop_scope="main"):
    import concourse.tile as tile
    from concourse import bacc, mybir
    from contextlib import ExitStack, nullcontext

    fp32 = mybir.dt.float32
    bf16 = mybir.dt.bfloat16
    fp8 = mybir.dt.float8e4
    AF = mybir.ActivationFunctionType
    AX = mybir.AxisListType
    DR = mybir.MatmulPerfMode.DoubleRow

    nc = bacc.Bacc("TRN2", target_bir_lowering=False, debug=False)
    q_in = nc.declare_dram_parameter("q_in", [C, QPC, HW], bf16, isOutput=False)
    s_in = nc.declare_dram_parameter("s_in", [C, NS, HW], bf16, isOutput=False)
    seg_in = nc.declare_dram_parameter("seg_in", [128, GROUPS, QPC], fp32, isOutput=False)
    score_out = nc.declare_dram_parameter("score_out", [QPC, WAY], fp32, isOutput=True)

    with ExitStack() as ctx:
        tc = ctx.enter_context(tile.TileContext(nc))
        const = ctx.enter_context(tc.tile_pool(name="const", bufs=1))
        sbig = ctx.enter_context(tc.tile_pool(name="sbig", bufs=1))
        stage = ctx.enter_context(tc.tile_pool(name="stage", bufs=2))
        qscr = ctx.enter_context(tc.tile_pool(name="qscr", bufs=6))
        small = ctx.enter_context(tc.tile_pool(name="small", bufs=2))
        t8p = ctx.enter_context(tc.tile_pool(name="t8p", bufs=8))
        psp = ctx.enter_context(tc.tile_pool(name="psp", bufs=8, space="PSUM"))

        # Constants
        ones_k = const.tile([128, 1], bf16, name="ones_k")
        nc.vector.memset(ones_k[:], 1.0)
        ones_m = const.tile([1, 128], bf16, name="ones_m")
        nc.vector.memset(ones_m[:], 1.0)

        seg = sbig.tile([128, GROUPS, QPC], fp32, name="seg")
        nc.sync.dma_start(out=seg[:], in_=seg_in[:])

        # ------------- loads (all fresh tiles; single-wait DMAs) -------------
        sn = []
        qn = []
        for kc in range(KC):
            qnk = sbig.tile([128, QPC, HW], bf16, name=f"qn{kc}")
            qn.append(qnk)
            nc.sync.dma_start(out=qnk[:], in_=q_in[kc * 128:(kc + 1) * 128])
            snk = sbig.tile([128, WAY, Y], bf16, name=f"sn{kc}")
            sn.append(snk)
            nc.sync.dma_start(
                out=snk[:].rearrange("c w (s x) -> c (w s) x", x=HW),
                in_=s_in[kc * 128:(kc + 1) * 128],
            )

        # fp8 DoubleRow pair layouts (pair strides 16-element aligned)
        q8p = [sbig.tile([128, 2, RPAD], fp8, name=f"q8p{i}") for i in range(2)]
        q8l = sbig.tile([128, ROWS], fp8, name="q8l")
        s8p = [sbig.tile([128, 2, WAY, YPAD], fp8, name=f"s8p{i}") for i in range(2)]
        s8l = sbig.tile([128, WAY, Y], fp8, name="s8l")

        # per-chunk persistent query-norm state
        ssqs = [sbig.tile([128, QPC], fp32, name=f"ssq{kc}") for kc in range(KC)]
        rqs = [sbig.tile([128, QPC], fp32, name=f"rq{kc}") for kc in range(KC)]

        body_cm = (
            tc.For_i(0, loop_reps, 1)
            if (loop_reps and loop_scope == "compute")
            else nullcontext()
        )
        with body_cm:
            if phases >= 2:
                def q_block(q0, q1):
                    """normalize + quantize queries [q0, q1) across all chunks"""
                    for kc in range(KC):
                        for q in range(q0, q1):
                            scr = qscr.tile([128, HW], bf16, name="scr")
                            nc.scalar.activation(
                                scr[:], qn[kc][:, q], AF.Square,
                                accum_out=ssqs[kc][:, q:q + 1],
                            )
                    for kc in range(KC):
                        nc.vector.reciprocal(
                            rqs[kc][:, q0:q1], ssqs[kc][:, q0:q1]
                        )
                        nc.scalar.activation(
                            rqs[kc][:, q0:q1], rqs[kc][:, q0:q1],
                            AF.Sqrt, scale=SQ * SQ,
                        )
                        q8_dst = (
                            q8p[kc // 2][:, kc % 2, 0:ROWS]
                            if kc < 4 else q8l[:]
                        ).rearrange("c (q x) -> c q x", x=HW)[:, q0:q1]
                        nc.gpsimd.tensor_mul(
                            q8_dst,
                            qn[kc][:, q0:q1],
                            rqs[kc][:, q0:q1].unsqueeze(2)
                                .broadcast_to([128, q1 - q0, HW]),
                        )

                # first query block gates the main loop: emit it first
                q_block(0, QBLOCKS[0])

                # support: squares + partition-reduce over c
                ss_t = [
                    psp.tile([1, BK], fp32, name=f"ss{yc}", tag="rel")
                    for yc in range(WAY)
                ]
                for kc in range(KC):
                    sq = stage.tile([128, YALL], bf16, name="sq", tag="sq")
                    s_flat = sn[kc][:].rearrange("c w y -> c (w y)")
                    nc.scalar.activation(sq[:], s_flat, AF.Square)
                    for yc in range(WAY):
                        nc.tensor.matmul(
                            ss_t[yc][:, 0:Y],
                            lhsT=ones_k[:],
                            rhs=sq[:, yc * Y:(yc + 1) * Y],
                            start=(kc == 0),
                            stop=(kc == KC - 1),
                        )
                # support finalize per way: w0 fully normalized first so the
                # main loop's w-order can start before later ways finish
                s_recip = small.tile([1, YALL], fp32, name="s_recip", bufs=1)
                s_rs = small.tile([1, YALL], bf16, name="s_rs", bufs=1)
                rs_sb = small.tile([128, WAY, Y], bf16, name="rs_sb", bufs=1)
                for yc in range(WAY):
                    nc.vector.reciprocal(
                        s_recip[:, yc * Y:(yc + 1) * Y], ss_t[yc][:, 0:Y]
                    )
                    nc.scalar.activation(
                        s_rs[:, yc * Y:(yc + 1) * Y],
                        s_recip[:, yc * Y:(yc + 1) * Y], AF.Sqrt, scale=SS * SS,
                    )
                    rb = psp.tile([128, BK], fp32, name=f"rs_bc{yc}", tag="rel")
                    nc.tensor.matmul(
                        rb[:, 0:Y],
                        lhsT=ones_m[:],
                        rhs=s_rs[:, yc * Y:(yc + 1) * Y],
                        start=True,
                        stop=True,
                    )
                    nc.scalar.copy(rs_sb[:, yc], rb[:, 0:Y])
                    for kc in range(KC):
                        s8_dst = (
                            s8p[kc // 2][:, kc % 2, yc, 0:Y]
                            if kc < 4 else s8l[:, yc]
                        )
                        eng = nc.vector if S_MUL_ENG[yc] == "V" else nc.gpsimd
                        eng.tensor_mul(s8_dst, sn[kc][:, yc], rs_sb[:, yc])

                # remaining query blocks stream behind the main loop's needs
                qa = QBLOCKS[0]
                for nb in QBLOCKS[1:]:
                    q_block(qa, qa + nb)
                    qa += nb

            if phases <= 2:
                score_sb = small.tile([QPC, WAY], fp32, name="score_sb")
                nc.vector.tensor_copy(score_sb[:], s8l[0:QPC, 0, 0:WAY])
                nc.sync.dma_start(out=score_out[:], in_=score_sb[:])

            # ------------- main loop: fp8 relation matmuls + top-8 -------------
            if phases >= 3:
                score_ps = psp.tile([QPC, WAY * 8], fp32, name="score_ps", tag="rel")
                loop_cm = (
                    tc.For_i(0, loop_reps, 1)
                    if (loop_reps and loop_scope == "main")
                    else nullcontext()
                )
                with loop_cm:
                    t8qs = [None] * GROUPS

                    def seg_mm(g):
                        m = min(128, ROWS - g * 128)
                        nc.tensor.matmul(
                            score_ps[:],
                            lhsT=seg[0:m, g],
                            rhs=t8qs[g][0:m],
                            start=(g == 0),
                            stop=(g == GROUPS - 1),
                        )

                    for g in range(GROUPS):
                        m = min(128, ROWS - g * 128)
                        t8q = t8p.tile([128, WAY * 8], fp32, name="t8q")
                        t8qs[g] = t8q
                        for w in range(WAY):
                            rel = psp.tile([128, Y], fp32, name="rel", tag="rel")
                            nc.tensor.matmul(
                                rel[0:m],
                                lhsT=q8p[0][:, :, g * 128:g * 128 + m],
                                rhs=s8p[0][:, :, w, 0:Y],
                                start=True, stop=False, perf_mode=DR,
                            )
                            nc.tensor.matmul(
                                rel[0:m],
                                lhsT=q8p[1][:, :, g * 128:g * 128 + m],
                                rhs=s8p[1][:, :, w, 0:Y],
                                start=False, stop=False, perf_mode=DR,
                            )
                            nc.tensor.matmul(
                                rel[0:m],
                                lhsT=q8l[:, g * 128:g * 128 + m],
                                rhs=s8l[:, w],
                                start=False, stop=True,
                            )
                            nc.vector.max(t8q[0:m, w * 8:(w + 1) * 8], rel[0:m])
                        if g >= SEG_DELAY:
                            seg_mm(g - SEG_DELAY)
                    for g in range(GROUPS - SEG_DELAY, GROUPS):
                        seg_mm(g)
                score_sb = small.tile([QPC, WAY], fp32, name="score_sb")
                nc.vector.reduce_sum(
                    score_sb[:],
                    score_ps[:].rearrange("q (w k) -> q w k", k=8)[:, :, 0:NK],
                    axis=AX.X,
                )
        if phases >= 3:
            nc.sync.dma_start(out=score_out[:], in_=score_sb[:])

    nc.compile()
    return nc


def _get_program():
    global _PROGRAM
    if _PROGRAM is None:
        _PROGRAM = _build_program()
    return _PROGRAM


def _seg_matrix():
    seg = np.zeros((128, GROUPS, QPC), dtype=np.float32)
    for r in range(ROWS):
        seg[r % 128, r // 128, r // HW] = 1.0 / (SQ * SS)
    return seg


def _make_in_maps(qf, sf):
    import ml_dtypes
    bf = ml_dtypes.bfloat16
    seg = _seg_matrix()
    in_maps = []
    for core in range(NCORES):
        t = core // 2
        q0 = 0 if core % 2 == 0 else WQ - QPC  # 0 or 37
        in_maps.append({
            "q_in": np.ascontiguousarray(
                qf[t, q0:q0 + QPC].transpose(1, 0, 2).astype(bf)),
            "s_in": np.ascontiguousarray(
                sf[t].transpose(1, 0, 2).astype(bf)),
            "seg_in": seg,
        })
    return in_maps


def kernel(query_feat, support_feat, way_num, shot_num, query_num, **_):
    from concourse.bass_utils import run_bass_kernel_spmd

    qf = np.asarray(query_feat, dtype=np.float32).reshape(T, WQ, C, HW)
    sf = np.asarray(support_feat, dtype=np.float32).reshape(T, NS, C, HW)
    assert int(way_num) == WAY and int(shot_num) == SHOT

    in_maps = _make_in_maps(qf, sf)
    res = run_bass_kernel_spmd(_get_program(), in_maps, list(range(NCORES))).results

    out = np.empty((T, WQ, WAY), dtype=np.float32)
    for t in range(T):
        lo = res[2 * t]["score_out"]
        hi = res[2 * t + 1]["score_out"]
        out[t, :QPC] = lo
        out[t, QPC:] = hi[QPC - (WQ - QPC):]  # drop the overlapping query row
    return out


# revision 26
# speedup vs baseline: 11.0622x; 11.0622x over previous
"""DN4 retrieval-kNN layer as a Trainium2 Bass/Tile kernel (fp8 DoubleRow).

Reference computation (shapes hardcoded from the problem spec):
  query_feat  [t=4, wq=75, c=640, 10, 10]  -> q normalized over hw axis (per (wq, c))
  support_feat[t=4, ws=25, c=640, 10, 10]  -> s normalized over c axis (per (way, y))
  relation[t, wq, way, x, y] = sum_c qn[t, wq, x, c] * sn[t, way, c, y]   (x=100, y=500)
  score[t, wq, way] = sum_x sum(top3_y(relation))

Sharding: 8 cores = 4 episodes (t) x 2 query-halves. Core 2t handles queries
[0:38), core 2t+1 handles queries [37:75) (38 rows each; query 37 is computed
twice and deduplicated on the host). No cross-device communication.

Device kernel (per core):
  - host prep: inputs pre-transposed to [c, n, x] bf16; the segment matrix for
    the per-query row sum carries the 1/(SQ*SS) fp8 descale.
  - query normalize: per-query ACT Square with accum_out gives sum-of-squares
    over hw without touching DVE; DVE reciprocal + ACT sqrt (fp8 scale folded
    in); GPSIMD multiplies apply the normalizer and emit fp8e4m3 directly into
    the DoubleRow pair layout. Queries are processed in blocks so the main
    loop can start after the first block.
  - support normalize: ACT squares, ones-matmul partition reduce (PE), DVE
    reciprocal + ACT sqrt, ones outer-product broadcast (PE), DVE/GPSIMD
    multiplies emit fp8.
  - main loop over 30 groups of 128 flattened (query, x) rows: per way, 2
    fp8 DoubleRow matmuls (256-deep contraction each) + 1 plain fp8 matmul
    accumulate the [128, 500] relation tile in PSUM at 2x bf16 throughput;
    DVE max8 extracts top-8 per row; a segment-matrix matmul (delayed two
    groups to keep the in-order PE queue from stalling on DVE) accumulates
    all 40 way/top8 lanes into PSUM; one final strided reduce sums top-3.
"""

import sys
import numpy as np

sys.path.insert(0, "/opt/trn_rl_repo")

T, WQ, C, HW = 4, 75, 640, 100
WAY, SHOT = 5, 5
NS = WAY * SHOT          # 25 support images per episode
Y = SHOT * HW            # 500 support descriptors per way
YALL = WAY * Y           # 2500
QPC = 38                 # queries per core (overlapping halves of 75)
KC = C // 128            # 5 contraction chunks of 128
NCORES = 8
NK = 3                   # top-k
ROWS = QPC * HW          # 3800 flattened (query, x) relation rows per core
GROUPS = (ROWS + 127) // 128   # 30 row-groups of <=128
BK = 512                 # PSUM bank stride in fp32 elements
SQ = 16.0                # fp8 scale on normalized query
SS = 16.0                # fp8 scale on normalized support
SEG_DELAY = 2            # groups to delay the seg matmul behind max8
QBLOCKS = [2, 2, 4, 6, 8, 8, 8]  # query pipeline blocks (sum = QPC)
QB_PRE = 2               # blocks emitted before the main loop
QB_MARGIN = 5            # groups of lead time for mid-loop block chains
QB_DVE = 2               # first N blocks use DVE square+reduce, not ACT accum
S_SQ_ENG = "V"           # support squares: V=DVE, A=ACT
S_MUL_ENG = "VVPPP"      # engine per contraction chunk for the support mul
RPAD = GROUPS * 128      # 3840: q8 pair stride must be 16-aligned (dual-fp8 ISA)
YPAD = 512               # s8 way stride, keeps the pair stride 16-aligned

_PROGRAM = None


def _build_program(phases=3, loop_reps=0, loop_scope="main", variant=""):
    import concourse.tile as tile
    from concourse import bacc, mybir
    from contextlib import ExitStack, nullcontext

    fp32 = mybir.dt.float32
    bf16 = mybir.dt.bfloat16
    fp8 = mybir.dt.float8e4
    AF = mybir.ActivationFunctionType
    AX = mybir.AxisListType
    DR = mybir.MatmulPerfMode.DoubleRow

    nc = bacc.Bacc("TRN2", target_bir_lowering=False, debug=False)
    q_in = nc.declare_dram_parameter("q_in", [C, QPC, HW], bf16, isOutput=False)
    s_in = nc.declare_dram_parameter("s_in", [C, NS, HW], bf16, isOutput=False)
    seg_in = nc.declare_dram_parameter("seg_in", [128, GROUPS, QPC], fp32, isOutput=False)
    score_out = nc.declare_dram_parameter("score_out", [QPC, WAY], fp32, isOutput=True)

    with ExitStack() as ctx:
        tc = ctx.enter_context(tile.TileContext(nc))
        const = ctx.enter_context(tc.tile_pool(name="const", bufs=1))
        sbig = ctx.enter_context(tc.tile_pool(name="sbig", bufs=1))
        stage = ctx.enter_context(tc.tile_pool(name="stage", bufs=2))
        qscr = ctx.enter_context(tc.tile_pool(name="qscr", bufs=6))
        small = ctx.enter_context(tc.tile_pool(name="small", bufs=2))
        t8p = ctx.enter_context(tc.tile_pool(name="t8p", bufs=8))
        psp = ctx.enter_context(tc.tile_pool(name="psp", bufs=8, space="PSUM"))

        # Constants
        ones_k = const.tile([128, 1], bf16, name="ones_k")
        nc.vector.memset(ones_k[:], 1.0)
        ones_m = const.tile([1, 128], bf16, name="ones_m")
        nc.vector.memset(ones_m[:], 1.0)
        warm_rhs = const.tile([128, BK], bf16, name="warm_rhs")
        nc.gpsimd.memset(warm_rhs[:], 1.0)

        seg = sbig.tile([128, GROUPS, QPC], fp32, name="seg")
        nc.sync.dma_start(out=seg[:], in_=seg_in[:])

        # ------------- loads (all fresh tiles; single-wait DMAs) -------------
        sn = []
        qn = []
        for kc in range(KC):
            qnk = sbig.tile([128, QPC, HW], bf16, name=f"qn{kc}")
            qn.append(qnk)
            nc.sync.dma_start(out=qnk[:], in_=q_in[kc * 128:(kc + 1) * 128])
            snk = sbig.tile([128, WAY, Y], bf16, name=f"sn{kc}")
            sn.append(snk)
            nc.sync.dma_start(
                out=snk[:].rearrange("c w (s x) -> c (w s) x", x=HW),
                in_=s_in[kc * 128:(kc + 1) * 128],
            )

        # fp8 DoubleRow pair layouts (pair strides 16-element aligned)
        q8p = [sbig.tile([128, 2, RPAD], fp8, name=f"q8p{i}") for i in range(2)]
        q8l = sbig.tile([128, ROWS], fp8, name="q8l")
        s8p = [sbig.tile([128, 2, WAY, YPAD], fp8, name=f"s8p{i}") for i in range(2)]
        s8l = sbig.tile([128, WAY, Y], fp8, name="s8l")

        # per-chunk persistent query-norm state
        ssqs = [sbig.tile([128, QPC], fp32, name=f"ssq{kc}") for kc in range(KC)]
        rqs = [sbig.tile([128, QPC], fp32, name=f"rq{kc}") for kc in range(KC)]

        body_cm = (
            tc.For_i(0, loop_reps, 1)
            if (loop_reps and loop_scope == "compute")
            else nullcontext()
        )
        with body_cm:
            # PE p-state warmup: ~2k cycles of dummy matmuls so the support
            # reduce and the first relation groups run at full clock
            warm_ps = psp.tile([1, BK], fp32, name="warm_ps", tag="rel")
            for i in range(10):
                nc.tensor.matmul(
                    warm_ps[:], lhsT=ones_k[:], rhs=warm_rhs[:],
                    start=(i == 0), stop=(i == 9),
                )
            if phases >= 2:
                def q_block(q0, q1, dve=False):
                    """normalize + quantize queries [q0, q1) across all chunks"""
                    if dve:
                        for kc in range(KC):
                            sqb = qscr.tile(
                                [128, 8 * HW], bf16, name="sqb", tag="sqb",
                            )[:, 0:(q1 - q0) * HW]
                            nc.vector.tensor_mul(
                                sqb,
                                qn[kc][:, q0:q1].rearrange("c q x -> c (q x)"),
                                qn[kc][:, q0:q1].rearrange("c q x -> c (q x)"),
                            )
                            nc.vector.reduce_sum(
                                ssqs[kc][:, q0:q1],
                                sqb.rearrange("c (q x) -> c q x", x=HW),
                                axis=AX.X,
                            )
                    else:
                        for kc in range(KC):
                            for q in range(q0, q1):
                                scr = qscr.tile([128, HW], bf16, name="scr")
                                nc.scalar.activation(
                                    scr[:], qn[kc][:, q], AF.Square,
                                    accum_out=ssqs[kc][:, q:q + 1],
                                )
                    for kc in range(KC):
                        nc.vector.reciprocal(
                            rqs[kc][:, q0:q1], ssqs[kc][:, q0:q1]
                        )
                        nc.scalar.activation(
                            rqs[kc][:, q0:q1], rqs[kc][:, q0:q1],
                            AF.Sqrt, scale=SQ * SQ,
                        )
                        q8_dst = (
                            q8p[kc // 2][:, kc % 2, 0:ROWS]
                            if kc < 4 else q8l[:]
                        ).rearrange("c (q x) -> c q x", x=HW)[:, q0:q1]
                        nc.gpsimd.tensor_mul(
                            q8_dst,
                            qn[kc][:, q0:q1],
                            rqs[kc][:, q0:q1].unsqueeze(2)
                                .broadcast_to([128, q1 - q0, HW]),
                        )

                # q-block schedule: first QB_PRE blocks before the main loop,
                # later blocks emitted mid-loop QB_MARGIN groups early
                qa = 0
                q_pre = []
                for b in range(QB_PRE):
                    q_pre.append((qa, qa + QBLOCKS[b], b < QB_DVE))
                    qa += QBLOCKS[b]
                q_sched = {}   # group -> (q0, q1, dve)
                for b in range(QB_PRE, len(QBLOCKS)):
                    g_need = (qa * HW) // 128
                    q_sched[max(0, g_need - QB_MARGIN)] = (
                        qa, qa + QBLOCKS[b], b < QB_DVE)
                    qa += QBLOCKS[b]

                # support squares while ACT runs q-accums
                sqs = []
                for kc in range(KC):
                    sq = stage.tile([128, YALL], bf16, name="sq", tag="sq",
                                    bufs=5)
                    s_flat = sn[kc][:].rearrange("c w y -> c (w y)")
                    if S_SQ_ENG == "V":
                        nc.vector.tensor_mul(sq[:], s_flat, s_flat)
                    else:
                        nc.scalar.activation(sq[:], s_flat, AF.Square)
                    sqs.append(sq)
                # first query block: overlaps the support squares
                q_block(*q_pre[0])
                # way-major partition-reduce so way 0 finalizes first
                ss_t = [
                    psp.tile([1, BK], fp32, name=f"ss{yc}", tag="rel")
                    for yc in range(WAY)
                ]
                for yc in range(WAY):
                    for kc in range(KC):
                        nc.tensor.matmul(
                            ss_t[yc][:, 0:Y],
                            lhsT=ones_k[:],
                            rhs=sqs[kc][:, yc * Y:(yc + 1) * Y],
                            start=(kc == 0),
                            stop=(kc == KC - 1),
                        )
                # per-way finalize: recip/sqrt/broadcast, then the scale-mul
                # for that way split across DVE (kc 0-2) and GPSIMD (kc 3-4)
                s_recip = small.tile([1, YALL], fp32, name="s_recip", bufs=1)
                s_rs = small.tile([1, YALL], bf16, name="s_rs", bufs=1)
                rs_sb = small.tile([128, WAY, Y], bf16, name="rs_sb", bufs=1)
                for yc in range(WAY):
                    nc.vector.reciprocal(
                        s_recip[:, yc * Y:(yc + 1) * Y], ss_t[yc][:, 0:Y]
                    )
                    nc.scalar.activation(
                        s_rs[:, yc * Y:(yc + 1) * Y],
                        s_recip[:, yc * Y:(yc + 1) * Y], AF.Sqrt, scale=SS * SS,
                    )
                    rb = psp.tile([128, BK], fp32, name=f"rs_bc{yc}", tag="rel")
                    nc.tensor.matmul(
                        rb[:, 0:Y],
                        lhsT=ones_m[:],
                        rhs=s_rs[:, yc * Y:(yc + 1) * Y],
                        start=True,
                        stop=True,
                    )
                    nc.scalar.copy(rs_sb[:, yc], rb[:, 0:Y])
                    for kc in range(KC):
                        s8_dst = (
                            s8p[kc // 2][:, kc % 2, yc, 0:Y]
                            if kc < 4 else s8l[:, yc]
                        )
                        eng = nc.vector if S_MUL_ENG[kc] == "V" else nc.gpsimd
                        eng.tensor_mul(s8_dst, sn[kc][:, yc], rs_sb[:, yc])
                # remaining pre-main query blocks
                for args in q_pre[1:]:
                    q_block(*args)

            if phases <= 2:
                score_sb = small.tile([QPC, WAY], fp32, name="score_sb")
                nc.vector.tensor_copy(score_sb[:], s8l[0:QPC, 0, 0:WAY])
                nc.sync.dma_start(out=score_out[:], in_=score_sb[:])

            # ------------- main loop: fp8 relation matmuls + top-8 -------------
            if phases >= 3:
                score_ps = psp.tile([QPC, WAY * 8], fp32, name="score_ps", tag="rel")
                loop_cm = (
                    tc.For_i(0, loop_reps, 1)
                    if (loop_reps and loop_scope == "main")
                    else nullcontext()
                )
                with loop_cm:
                    t8qs = [None] * GROUPS
                    rel_dummy = None
                    if variant == "nomm":
                        rel_dummy = psp.tile([128, Y], fp32, name="rel_d", tag="rel")
                        nc.vector.memset(rel_dummy[:], 0.5)

                    def seg_mm(g):
                        m = min(128, ROWS - g * 128)
                        nc.tensor.matmul(
                            score_ps[:],
                            lhsT=seg[0:m, g],
                            rhs=t8qs[g][0:m],
                            start=(g == 0),
                            stop=(g == GROUPS - 1),
                        )

                    for g in range(GROUPS):
                        if phases >= 2 and g in q_sched:
                            q_block(*q_sched[g])
                        m = min(128, ROWS - g * 128)
                        t8q = t8p.tile([128, WAY * 8], fp32, name="t8q")
                        t8qs[g] = t8q
                        for w in range(WAY):
                            rel = (
                                rel_dummy if variant == "nomm"
                                else psp.tile([128, Y], fp32, name="rel", tag="rel")
                            )
                            if variant != "nomm":
                                nc.tensor.matmul(
                                    rel[0:m],
                                    lhsT=q8p[0][:, :, g * 128:g * 128 + m],
                                    rhs=s8p[0][:, :, w, 0:Y],
                                    start=True, stop=False, perf_mode=DR,
                                )
                                nc.tensor.matmul(
                                    rel[0:m],
                                    lhsT=q8p[1][:, :, g * 128:g * 128 + m],
                                    rhs=s8p[1][:, :, w, 0:Y],
                                    start=False, stop=False, perf_mode=DR,
                                )
                                nc.tensor.matmul(
                                    rel[0:m],
                                    lhsT=q8l[:, g * 128:g * 128 + m],
                                    rhs=s8l[:, w],
                                    start=False, stop=True,
                                )
                            if variant != "nomax":
                                nc.vector.max(
                                    t8q[0:m, w * 8:(w + 1) * 8], rel[0:m]
                                )
                        if variant != "nomax":
                            if g >= SEG_DELAY:
                                seg_mm(g - SEG_DELAY)
                    if variant != "nomax":
                        for g in range(GROUPS - SEG_DELAY, GROUPS):
                            seg_mm(g)
                score_sb = small.tile([QPC, WAY], fp32, name="score_sb")
                if variant == "nomax":
                    nc.vector.memset(score_sb[:], 0.0)
                else:
                    nc.vector.reduce_sum(
                        score_sb[:],
                        score_ps[:].rearrange("q (w k) -> q w k", k=8)[:, :, 0:NK],
                        axis=AX.X,
                    )
        if phases >= 3:
            nc.sync.dma_start(out=score_out[:], in_=score_sb[:])

    nc.compile()
    return nc


def _get_program():
    global _PROGRAM
    if _PROGRAM is None:
        _PROGRAM = _build_program()
    return _PROGRAM


def _seg_matrix():
    seg = np.zeros((128, GROUPS, QPC), dtype=np.float32)
    for r in range(ROWS):
        seg[r % 128, r // 128, r // HW] = 1.0 / (SQ * SS)
    return seg


def _make_in_maps(qf, sf):
    import ml_dtypes
    bf = ml_dtypes.bfloat16
    seg = _seg_matrix()
    in_maps = []
    for core in range(NCORES):
        t = core // 2
        q0 = 0 if core % 2 == 0 else WQ - QPC  # 0 or 37
        in_maps.append({
            "q_in": np.ascontiguousarray(
                qf[t, q0:q0 + QPC].transpose(1, 0, 2).astype(bf)),
            "s_in": np.ascontiguousarray(
                sf[t].transpose(1, 0, 2).astype(bf)),
            "seg_in": seg,
        })
    return in_maps


def kernel(query_feat, support_feat, way_num, shot_num, query_num, **_):
    from concourse.bass_utils import run_bass_kernel_spmd

    qf = np.asarray(query_feat, dtype=np.float32).reshape(T, WQ, C, HW)
    sf = np.asarray(support_feat, dtype=np.float32).reshape(T, NS, C, HW)
    assert int(way_num) == WAY and int(shot_num) == SHOT

    in_maps = _make_in_maps(qf, sf)
    res = run_bass_kernel_spmd(_get_program(), in_maps, list(range(NCORES))).results

    out = np.empty((T, WQ, WAY), dtype=np.float32)
    for t in range(T):
        lo = res[2 * t]["score_out"]
        hi = res[2 * t + 1]["score_out"]
        out[t, :QPC] = lo
        out[t, QPC:] = hi[QPC - (WQ - QPC):]  # drop the overlapping query row
    return out


# revision 27
# speedup vs baseline: 11.5572x; 1.0448x over previous
"""DN4 retrieval-kNN layer as a Trainium2 Bass/Tile kernel (fp8 DoubleRow).

Reference computation (shapes hardcoded from the problem spec):
  query_feat  [t=4, wq=75, c=640, 10, 10]  -> q normalized over hw axis (per (wq, c))
  support_feat[t=4, ws=25, c=640, 10, 10]  -> s normalized over c axis (per (way, y))
  relation[t, wq, way, x, y] = sum_c qn[t, wq, x, c] * sn[t, way, c, y]   (x=100, y=500)
  score[t, wq, way] = sum_x sum(top3_y(relation))

Sharding: 8 cores = 4 episodes (t) x 2 query-halves. Core 2t handles queries
[0:38), core 2t+1 handles queries [37:75) (38 rows each; query 37 is computed
twice and deduplicated on the host). No cross-device communication.

Device kernel (per core):
  - host prep: inputs pre-transposed to [c, n, x] bf16; the segment matrix for
    the per-query row sum carries the 1/(SQ*SS) fp8 descale.
  - query normalize: per-query ACT Square with accum_out gives sum-of-squares
    over hw without touching DVE; DVE reciprocal + ACT sqrt (fp8 scale folded
    in); GPSIMD multiplies apply the normalizer and emit fp8e4m3 directly into
    the DoubleRow pair layout. Queries are processed in blocks so the main
    loop can start after the first block.
  - support normalize: ACT squares, ones-matmul partition reduce (PE), DVE
    reciprocal + ACT sqrt, ones outer-product broadcast (PE), DVE/GPSIMD
    multiplies emit fp8.
  - main loop over 30 groups of 128 flattened (query, x) rows: per way, 2
    fp8 DoubleRow matmuls (256-deep contraction each) + 1 plain fp8 matmul
    accumulate the [128, 500] relation tile in PSUM at 2x bf16 throughput;
    DVE max8 extracts top-8 per row; a segment-matrix matmul (delayed two
    groups to keep the in-order PE queue from stalling on DVE) accumulates
    all 40 way/top8 lanes into PSUM; one final strided reduce sums top-3.
"""

import sys
import numpy as np

sys.path.insert(0, "/opt/trn_rl_repo")

T, WQ, C, HW = 4, 75, 640, 100
WAY, SHOT = 5, 5
NS = WAY * SHOT          # 25 support images per episode
Y = SHOT * HW            # 500 support descriptors per way
YALL = WAY * Y           # 2500
QPC = 38                 # queries per core (overlapping halves of 75)
KC = C // 128            # 5 contraction chunks of 128
NCORES = 8
NK = 3                   # top-k
ROWS = QPC * HW          # 3800 flattened (query, x) relation rows per core
GROUPS = (ROWS + 127) // 128   # 30 row-groups of <=128
BK = 512                 # PSUM bank stride in fp32 elements
SQ = 16.0                # fp8 scale on normalized query
SS = 16.0                # fp8 scale on normalized support
SEG_DELAY = 2            # groups to delay the seg matmul behind max8
QBLOCKS = [2, 2, 4, 6, 8, 8, 8]  # query pipeline blocks (sum = QPC)
QB_PRE = 2               # blocks emitted before the main loop
QB_MARGIN = 5            # groups of lead time for mid-loop block chains
QB_DVE = 2               # first N blocks use DVE square+reduce, not ACT accum
S_SQ_ENG = "V"           # support squares: V=DVE, A=ACT
S_MUL_ENG = "VVPPP"      # engine per contraction chunk for the support mul
RPAD = GROUPS * 128      # 3840: q8 pair stride must be 16-aligned (dual-fp8 ISA)
YPAD = 512               # s8 way stride, keeps the pair stride 16-aligned

_PROGRAM = None


def _build_program(phases=3, loop_reps=0, loop_scope="main", variant=""):
    import concourse.tile as tile
    from concourse import bacc, mybir
    from contextlib import ExitStack, nullcontext

    fp32 = mybir.dt.float32
    bf16 = mybir.dt.bfloat16
    fp8 = mybir.dt.float8e4
    AF = mybir.ActivationFunctionType
    AX = mybir.AxisListType
    DR = mybir.MatmulPerfMode.DoubleRow

    nc = bacc.Bacc("TRN2", target_bir_lowering=False, debug=False)
    q_in = nc.declare_dram_parameter("q_in", [C, QPC, HW], bf16, isOutput=False)
    s_in = nc.declare_dram_parameter("s_in", [C, NS, HW], bf16, isOutput=False)
    seg_in = nc.declare_dram_parameter("seg_in", [128, GROUPS, QPC], fp32, isOutput=False)
    score_out = nc.declare_dram_parameter("score_out", [QPC, WAY], fp32, isOutput=True)

    with ExitStack() as ctx:
        tc = ctx.enter_context(tile.TileContext(nc))
        const = ctx.enter_context(tc.tile_pool(name="const", bufs=1))
        sbig = ctx.enter_context(tc.tile_pool(name="sbig", bufs=1))
        stage = ctx.enter_context(tc.tile_pool(name="stage", bufs=2))
        qscr = ctx.enter_context(tc.tile_pool(name="qscr", bufs=6))
        small = ctx.enter_context(tc.tile_pool(name="small", bufs=2))
        t8p = ctx.enter_context(tc.tile_pool(name="t8p", bufs=8))
        psp = ctx.enter_context(tc.tile_pool(name="psp", bufs=8, space="PSUM"))

        # Constants
        ones_k = const.tile([128, 1], bf16, name="ones_k")
        nc.vector.memset(ones_k[:], 1.0)
        ones_m = const.tile([1, 128], bf16, name="ones_m")
        nc.vector.memset(ones_m[:], 1.0)
        warm_rhs = const.tile([128, BK], bf16, name="warm_rhs")
        nc.gpsimd.memset(warm_rhs[:], 1.0)

        # ------------- loads: q-head + s first (gate the normalize chains),
        # q-tail + seg stream behind them -------------
        QH = 8   # queries covered by the pre-main blocks
        sn = []
        qn = []
        for kc in range(KC):
            qnk = sbig.tile([128, QPC, HW], bf16, name=f"qn{kc}")
            qn.append(qnk)
            snk = sbig.tile([128, WAY, Y], bf16, name=f"sn{kc}")
            sn.append(snk)
        for kc in range(KC):
            nc.sync.dma_start(
                out=qn[kc][:, 0:QH], in_=q_in[kc * 128:(kc + 1) * 128, 0:QH]
            )
        for kc in range(KC):
            nc.sync.dma_start(
                out=sn[kc][:].rearrange("c w (s x) -> c (w s) x", x=HW),
                in_=s_in[kc * 128:(kc + 1) * 128],
            )
        for kc in range(KC):
            nc.sync.dma_start(
                out=qn[kc][:, QH:QPC], in_=q_in[kc * 128:(kc + 1) * 128, QH:QPC]
            )
        seg = sbig.tile([128, GROUPS, QPC], fp32, name="seg")
        nc.sync.dma_start(out=seg[:], in_=seg_in[:])

        # fp8 DoubleRow pair layouts (pair strides 16-element aligned)
        q8p = [sbig.tile([128, 2, RPAD], fp8, name=f"q8p{i}") for i in range(2)]
        q8l = sbig.tile([128, ROWS], fp8, name="q8l")
        s8p = [sbig.tile([128, 2, WAY, YPAD], fp8, name=f"s8p{i}") for i in range(2)]
        s8l = sbig.tile([128, WAY, Y], fp8, name="s8l")

        # per-chunk persistent query-norm state
        ssqs = [sbig.tile([128, QPC], fp32, name=f"ssq{kc}") for kc in range(KC)]
        rqs = [sbig.tile([128, QPC], fp32, name=f"rq{kc}") for kc in range(KC)]

        body_cm = (
            tc.For_i(0, loop_reps, 1)
            if (loop_reps and loop_scope == "compute")
            else nullcontext()
        )
        with body_cm:
            # PE p-state warmup: ~2k cycles of dummy matmuls so the support
            # reduce and the first relation groups run at full clock
            warm_ps = psp.tile([1, BK], fp32, name="warm_ps", tag="rel")
            for i in range(10):
                nc.tensor.matmul(
                    warm_ps[:], lhsT=ones_k[:], rhs=warm_rhs[:],
                    start=(i == 0), stop=(i == 9),
                )
            if phases >= 2:
                def q_block(q0, q1, dve=False):
                    """normalize + quantize queries [q0, q1) across all chunks"""
                    if dve:
                        for kc in range(KC):
                            sqb = qscr.tile(
                                [128, 8 * HW], bf16, name="sqb", tag="sqb",
                            )[:, 0:(q1 - q0) * HW]
                            nc.vector.tensor_mul(
                                sqb,
                                qn[kc][:, q0:q1].rearrange("c q x -> c (q x)"),
                                qn[kc][:, q0:q1].rearrange("c q x -> c (q x)"),
                            )
                            nc.vector.reduce_sum(
                                ssqs[kc][:, q0:q1],
                                sqb.rearrange("c (q x) -> c q x", x=HW),
                                axis=AX.X,
                            )
                    else:
                        for kc in range(KC):
                            for q in range(q0, q1):
                                scr = qscr.tile([128, HW], bf16, name="scr")
                                nc.scalar.activation(
                                    scr[:], qn[kc][:, q], AF.Square,
                                    accum_out=ssqs[kc][:, q:q + 1],
                                )
                    for kc in range(KC):
                        nc.vector.reciprocal(
                            rqs[kc][:, q0:q1], ssqs[kc][:, q0:q1]
                        )
                        nc.scalar.activation(
                            rqs[kc][:, q0:q1], rqs[kc][:, q0:q1],
                            AF.Sqrt, scale=SQ * SQ,
                        )
                        q8_dst = (
                            q8p[kc // 2][:, kc % 2, 0:ROWS]
                            if kc < 4 else q8l[:]
                        ).rearrange("c (q x) -> c q x", x=HW)[:, q0:q1]
                        nc.gpsimd.tensor_mul(
                            q8_dst,
                            qn[kc][:, q0:q1],
                            rqs[kc][:, q0:q1].unsqueeze(2)
                                .broadcast_to([128, q1 - q0, HW]),
                        )

                # q-block schedule: first QB_PRE blocks before the main loop,
                # later blocks emitted mid-loop QB_MARGIN groups early
                qa = 0
                q_pre = []
                for b in range(QB_PRE):
                    q_pre.append((qa, qa + QBLOCKS[b], b < QB_DVE))
                    qa += QBLOCKS[b]
                q_sched = {}   # group -> (q0, q1, dve)
                for b in range(QB_PRE, len(QBLOCKS)):
                    g_need = (qa * HW) // 128
                    q_sched[max(0, g_need - QB_MARGIN)] = (
                        qa, qa + QBLOCKS[b], b < QB_DVE)
                    qa += QBLOCKS[b]

                # support squares while ACT runs q-accums
                sqs = []
                for kc in range(KC):
                    sq = stage.tile([128, YALL], bf16, name="sq", tag="sq",
                                    bufs=5)
                    s_flat = sn[kc][:].rearrange("c w y -> c (w y)")
                    if S_SQ_ENG == "V":
                        nc.vector.tensor_mul(sq[:], s_flat, s_flat)
                    else:
                        nc.scalar.activation(sq[:], s_flat, AF.Square)
                    sqs.append(sq)
                # first query block: overlaps the support squares
                q_block(*q_pre[0])
                # way-major partition-reduce so way 0 finalizes first
                ss_t = [
                    psp.tile([1, BK], fp32, name=f"ss{yc}", tag="rel")
                    for yc in range(WAY)
                ]
                for yc in range(WAY):
                    for kc in range(KC):
                        nc.tensor.matmul(
                            ss_t[yc][:, 0:Y],
                            lhsT=ones_k[:],
                            rhs=sqs[kc][:, yc * Y:(yc + 1) * Y],
                            start=(kc == 0),
                            stop=(kc == KC - 1),
                        )
                # per-way finalize: recip/sqrt/broadcast, then the scale-mul
                # for that way split across DVE (kc 0-2) and GPSIMD (kc 3-4)
                s_recip = small.tile([1, YALL], fp32, name="s_recip", bufs=1)
                s_rs = small.tile([1, YALL], bf16, name="s_rs", bufs=1)
                rs_sb = small.tile([128, WAY, Y], bf16, name="rs_sb", bufs=1)
                for yc in range(WAY):
                    nc.vector.reciprocal(
                        s_recip[:, yc * Y:(yc + 1) * Y], ss_t[yc][:, 0:Y]
                    )
                    nc.scalar.activation(
                        s_rs[:, yc * Y:(yc + 1) * Y],
                        s_recip[:, yc * Y:(yc + 1) * Y], AF.Sqrt, scale=SS * SS,
                    )
                    rb = psp.tile([128, BK], fp32, name=f"rs_bc{yc}", tag="rel")
                    nc.tensor.matmul(
                        rb[:, 0:Y],
                        lhsT=ones_m[:],
                        rhs=s_rs[:, yc * Y:(yc + 1) * Y],
                        start=True,
                        stop=True,
                    )
                    nc.scalar.copy(rs_sb[:, yc], rb[:, 0:Y])
                    for kc in range(KC):
                        s8_dst = (
                            s8p[kc // 2][:, kc % 2, yc, 0:Y]
                            if kc < 4 else s8l[:, yc]
                        )
                        eng = nc.vector if S_MUL_ENG[kc] == "V" else nc.gpsimd
                        eng.tensor_mul(s8_dst, sn[kc][:, yc], rs_sb[:, yc])
                # remaining pre-main query blocks
                for args in q_pre[1:]:
                    q_block(*args)

            if phases <= 2:
                score_sb = small.tile([QPC, WAY], fp32, name="score_sb")
                nc.vector.tensor_copy(score_sb[:], s8l[0:QPC, 0, 0:WAY])
                nc.sync.dma_start(out=score_out[:], in_=score_sb[:])

            # ------------- main loop: fp8 relation matmuls + top-8 -------------
            if phases >= 3:
                score_ps = psp.tile([QPC, WAY * 8], fp32, name="score_ps", tag="rel")
                loop_cm = (
                    tc.For_i(0, loop_reps, 1)
                    if (loop_reps and loop_scope == "main")
                    else nullcontext()
                )
                with loop_cm:
                    t8qs = [None] * GROUPS
                    rel_dummy = None
                    if variant == "nomm":
                        rel_dummy = psp.tile([128, Y], fp32, name="rel_d", tag="rel")
                        nc.vector.memset(rel_dummy[:], 0.5)

                    def seg_mm(g):
                        m = min(128, ROWS - g * 128)
                        nc.tensor.matmul(
                            score_ps[:],
                            lhsT=seg[0:m, g],
                            rhs=t8qs[g][0:m],
                            start=(g == 0),
                            stop=(g == GROUPS - 1),
                        )

                    for g in range(GROUPS):
                        if phases >= 2 and g in q_sched:
                            q_block(*q_sched[g])
                        m = min(128, ROWS - g * 128)
                        t8q = t8p.tile([128, WAY * 8], fp32, name="t8q")
                        t8qs[g] = t8q
                        for w in range(WAY):
                            rel = (
                                rel_dummy if variant == "nomm"
                                else psp.tile([128, Y], fp32, name="rel", tag="rel")
                            )
                            if variant != "nomm":
                                nc.tensor.matmul(
                                    rel[0:m],
                                    lhsT=q8p[0][:, :, g * 128:g * 128 + m],
                                    rhs=s8p[0][:, :, w, 0:Y],
                                    start=True, stop=False, perf_mode=DR,
                                )
                                nc.tensor.matmul(
                                    rel[0:m],
                                    lhsT=q8p[1][:, :, g * 128:g * 128 + m],
                                    rhs=s8p[1][:, :, w, 0:Y],
                                    start=False, stop=False, perf_mode=DR,
                                )
                                nc.tensor.matmul(
                                    rel[0:m],
                                    lhsT=q8l[:, g * 128:g * 128 + m],
                                    rhs=s8l[:, w],
                                    start=False, stop=True,
                                )
                            if variant != "nomax":
                                nc.vector.max(
                                    t8q[0:m, w * 8:(w + 1) * 8], rel[0:m]
                                )
                        if variant != "nomax":
                            if g >= SEG_DELAY:
                                seg_mm(g - SEG_DELAY)
                    if variant != "nomax":
                        for g in range(GROUPS - SEG_DELAY, GROUPS):
                            seg_mm(g)
                score_sb = small.tile([QPC, WAY], fp32, name="score_sb")
                if variant == "nomax":
                    nc.vector.memset(score_sb[:], 0.0)
                else:
                    nc.vector.reduce_sum(
                        score_sb[:],
                        score_ps[:].rearrange("q (w k) -> q w k", k=8)[:, :, 0:NK],
                        axis=AX.X,
                    )
        if phases >= 3:
            nc.sync.dma_start(out=score_out[:], in_=score_sb[:])

    nc.compile()
    return nc


def _get_program():
    global _PROGRAM
    if _PROGRAM is None:
        _PROGRAM = _build_program()
    return _PROGRAM


def _seg_matrix():
    seg = np.zeros((128, GROUPS, QPC), dtype=np.float32)
    for r in range(ROWS):
        seg[r % 128, r // 128, r // HW] = 1.0 / (SQ * SS)
    return seg


def _make_in_maps(qf, sf):
    import ml_dtypes
    bf = ml_dtypes.bfloat16
    seg = _seg_matrix()
    in_maps = []
    for core in range(NCORES):
        t = core // 2
        q0 = 0 if core % 2 == 0 else WQ - QPC  # 0 or 37
        in_maps.append({
            "q_in": np.ascontiguousarray(
                qf[t, q0:q0 + QPC].transpose(1, 0, 2).astype(bf)),
            "s_in": np.ascontiguousarray(
                sf[t].transpose(1, 0, 2).astype(bf)),
            "seg_in": seg,
        })
    return in_maps


def kernel(query_feat, support_feat, way_num, shot_num, query_num, **_):
    from concourse.bass_utils import run_bass_kernel_spmd

    qf = np.asarray(query_feat, dtype=np.float32).reshape(T, WQ, C, HW)
    sf = np.asarray(support_feat, dtype=np.float32).reshape(T, NS, C, HW)
    assert int(way_num) == WAY and int(shot_num) == SHOT

    in_maps = _make_in_maps(qf, sf)
    res = run_bass_kernel_spmd(_get_program(), in_maps, list(range(NCORES))).results

    out = np.empty((T, WQ, WAY), dtype=np.float32)
    for t in range(T):
        lo = res[2 * t]["score_out"]
        hi = res[2 * t + 1]["score_out"]
        out[t, :QPC] = lo
        out[t, QPC:] = hi[QPC - (WQ - QPC):]  # drop the overlapping query row
    return out
